# revision 13
# baseline (speedup 1.0000x reference)
import sys
import os

sys.path.insert(0, "/opt/trn_rl_repo")

import numpy as np
import math

import concourse.bass as bass
import concourse.tile as tile
from concourse import bacc, mybir
from concourse.bass_utils import run_bass_kernel_spmd

F32 = mybir.dt.float32
I32 = mybir.dt.int32
AF = mybir.ActivationFunctionType
ALU = mybir.AluOpType

DIM = 512
IN_OUT = 768
DEPTH = 4
HEADS = 8
DIM_HEAD = 64
INNER = 512
PFD = 256
FF_MULT = 4
ROT_DIM = 32
NUM_BUCKETS = 32
MAX_DIST = 128
B, N, M = 4, 1024, 1024
NLAYERS = int(os.environ.get("KB_NLAYERS", str(DEPTH + 2)))
R = 512
P = 128
NCORES = 8
JT = 9
EPS = 1e-5
MMDT = getattr(mybir.dt, os.environ.get("KB_MMDT", "float32"))


def _layer_cfgs():
    cfgs = [dict(self_in=IN_OUT, d=DIM, chan_out=DIM, ff_d=DIM)]
    for _ in range(DEPTH):
        cfgs.append(dict(self_in=DIM, d=DIM, chan_out=DIM, ff_d=DIM))
    cfgs.append(dict(self_in=DIM, d=DIM, chan_out=IN_OUT, ff_d=IN_OUT))
    return cfgs


CFGS = _layer_cfgs()[:NLAYERS]
DLAST = IN_OUT if NLAYERS == DEPTH + 2 else CFGS[-1]["d"]



def _bucket_table():
    neg = np.arange(0, 2050)
    max_exact = NUM_BUCKETS // 2
    is_small = neg < max_exact
    large = max_exact + (
        np.log(np.maximum(neg, 1) / max_exact)
        / math.log(MAX_DIST / max_exact)
        * (NUM_BUCKETS - max_exact)
    ).astype(np.int64)
    large = np.minimum(large, NUM_BUCKETS - 1)
    return np.where(is_small, neg, large)


def make_bias_tables(emb, parity):
    bt = _bucket_table()
    emb = np.asarray(emb, np.float64)
    tabs = np.zeros((HEADS, 2, 2048), np.float32)
    s = np.arange(2048)
    d = s - 1023 + 512 * parity
    valid = (d >= 0) & (d < N)
    negidx = np.clip(np.maximum(d - 1, 0), 0, 2049)
    u = np.arange(2048) + 512 * parity
    uvalid = u < N
    for h in range(HEADS):
        vals = np.exp(emb[bt[negidx], h])
        tabs[h, 0, :] = np.where(valid, vals, 0.0)
        tabs[h, 1, :] = np.where(uvalid, np.exp(emb[bt[np.clip(u, 0, 2049)], h]), 0.0)
    return tabs


def make_rotary(parity):
    inv_freq = 1.0 / (10000.0 ** (np.arange(0, ROT_DIM, 2, dtype=np.float32) / ROT_DIM))
    pos_all = np.arange(N, dtype=np.float32)[:, None] * inv_freq[None, :].astype(np.float32)
    pos_all = np.concatenate([pos_all, pos_all], axis=1)
    rows = np.arange(R) + R * parity
    rows_rev = (R - 1 - np.arange(R)) + R * parity
    cosq = np.ones((P, R), np.float32)
    sinq = np.zeros((P, R), np.float32)
    for p in range(P):
        dd = p % DIM_HEAD
        if dd < ROT_DIM:
            cosq[p, :] = np.cos(pos_all[rows, dd])
            sinq[p, :] = np.sin(pos_all[rows, dd])
    cosk = np.ones((DIM_HEAD, R), np.float32)
    sink = np.zeros((DIM_HEAD, R), np.float32)
    for dd in range(DIM_HEAD):
        if dd < ROT_DIM:
            cosk[dd, :] = np.cos(pos_all[rows_rev, dd])
            sink[dd, :] = np.sin(pos_all[rows_rev, dd])
    return cosq, sinq, cosk, sink


def _sigma(w):
    ws = np.zeros_like(w)
    nh = w.shape[1] // DIM_HEAD
    for h in range(nh):
        b = h * DIM_HEAD
        ws[:, b:b + 16] = -w[:, b + 16:b + 32]
        ws[:, b + 16:b + 32] = w[:, b:b + 16]
    return ws


def fold_weights(params):
    scale = DIM_HEAD ** -0.5
    out = []
    for lp in params["layers"][:NLAYERS]:
        L = {}
        sp = lp["self"]
        g = np.asarray(sp["norm_g"], np.float32)
        wq = np.asarray(sp["wq"], np.float32) * g[:, None] * scale
        wkv = np.asarray(sp["wkv"], np.float32) * g[:, None]
        L["wq"] = wq
        L["wqs"] = _sigma(wq)
        L["wk"] = np.ascontiguousarray(wkv[:, :DIM_HEAD])
        L["wks"] = _sigma(L["wk"])
        L["wv"] = np.ascontiguousarray(wkv[:, DIM_HEAD:])
        L["null_k"] = np.ascontiguousarray(np.asarray(sp["null_kv"], np.float32)[0])
        L["null_v"] = np.ascontiguousarray(np.asarray(sp["null_kv"], np.float32)[1])
        L["wo_self"] = np.asarray(sp["wo"], np.float32)
        L["g_self_out"] = np.asarray(sp["out_norm_g"], np.float32)
        cp = lp["cross"]
        g = np.asarray(cp["norm_g"], np.float32)
        L["xwq"] = np.asarray(cp["wq"], np.float32) * g[:, None] * scale
        gc = np.asarray(cp["ctx_norm_g"], np.float32)
        xwkv = np.asarray(cp["wkv"], np.float32) * gc[:, None]
        L["xwk"] = np.ascontiguousarray(xwkv[:, :DIM_HEAD])
        L["xwv"] = np.ascontiguousarray(xwkv[:, DIM_HEAD:])
        L["xnull_k"] = np.ascontiguousarray(np.asarray(cp["null_kv"], np.float32)[0])
        L["xnull_v"] = np.ascontiguousarray(np.asarray(cp["null_kv"], np.float32)[1])
        L["wo_cross"] = np.asarray(cp["wo"], np.float32)
        L["g_cross_out"] = np.asarray(cp["out_norm_g"], np.float32)
        hp = lp["chan"]
        g = np.asarray(hp["norm_g"], np.float32)
        wqkv = (np.asarray(hp["wqkv"], np.float32) * g[:, None]).copy()
        wqkv[:, :INNER] *= scale
        L["cwqkv"] = wqkv
        L["wo_chan"] = np.asarray(hp["wo"], np.float32)
        L["g_chan_out"] = np.asarray(hp["out_norm_g"], np.float32)
        fp = lp["ff"]
        g = np.asarray(fp["norm_g"], np.float32)
        L["w1"] = np.asarray(fp["w1"], np.float32) * g[:, None]
        L["w2"] = np.asarray(fp["w2"], np.float32)
        out.append(L)
    return out



WNAMES_MM = ["wq", "wqs", "wk", "wks", "wv", "wo_self", "xwq", "xwk", "xwv",
             "wo_cross", "cwqkv", "wo_chan", "w1", "w2"]


def build_program():
    nc = bacc.Bacc(None, target_bir_lowering=False, num_devices=NCORES)
    groups = [[0, 1], [2, 3], [4, 5], [6, 7]]

    din0 = CFGS[0]["self_in"]
    T = {}
    T["x_in"] = nc.dram_tensor("x_in", [R, din0], F32, kind="ExternalInput")
    T["ctx_in"] = nc.dram_tensor("ctx_in", [M, PFD], F32, kind="ExternalInput")
    T["y_out"] = nc.dram_tensor("y_out", [R, DLAST], F32, kind="ExternalOutput")
    T["rev_in"] = nc.dram_tensor("rev_in", [P, P], F32, kind="ExternalInput")
    T["ident_in"] = nc.dram_tensor("ident_in", [P, P], F32, kind="ExternalInput")
    T["mask_in"] = nc.dram_tensor("mask_in", [2], F32, kind="ExternalInput")
    T["cosq_in"] = nc.dram_tensor("cosq_in", [P, R], F32, kind="ExternalInput")
    T["sinq_in"] = nc.dram_tensor("sinq_in", [P, R], F32, kind="ExternalInput")
    T["cosk_in"] = nc.dram_tensor("cosk_in", [DIM_HEAD, R], F32, kind="ExternalInput")
    T["sink_in"] = nc.dram_tensor("sink_in", [DIM_HEAD, R], F32, kind="ExternalInput")
    T["bias_in"] = nc.dram_tensor("bias_in", [HEADS, 2, 2048], F32, kind="ExternalInput")
    T["gfin_in"] = nc.dram_tensor("gfin_in", [DLAST], F32, kind="ExternalInput")
    T["dbg_out"] = nc.dram_tensor("dbg_out", [P, 8192], F32, kind="ExternalOutput")

    W = []
    for li, c in enumerate(CFGS):
        din, d, dco, dff = c["self_in"], c["d"], c["chan_out"], c["ff_d"]
        inner = FF_MULT * dff
        shp = dict(wq=[din, INNER], wqs=[din, INNER], wk=[din, DIM_HEAD],
                   wks=[din, DIM_HEAD], wv=[din, DIM_HEAD], wo_self=[INNER, d],
                   xwq=[d, INNER], xwk=[PFD, DIM_HEAD], xwv=[PFD, DIM_HEAD],
                   wo_cross=[INNER, d], cwqkv=[d, 3 * INNER], wo_chan=[INNER, dco],
                   w1=[dff, 2 * inner], w2=[inner, dff],
                   null_k=[DIM_HEAD], null_v=[DIM_HEAD],
                   xnull_k=[DIM_HEAD], xnull_v=[DIM_HEAD],
                   g_self_out=[d], g_cross_out=[d], g_chan_out=[dco])
        wl = {nm: nc.dram_tensor(f"L{li}_{nm}", s,
                                 MMDT if nm in WNAMES_MM else F32,
                                 kind="ExternalInput")
              for nm, s in shp.items()}
        W.append(wl)
    T["W"] = W

    KVLEN = 64 * R + R * 64
    T["KVLEN"] = KVLEN
    T["kv_cc_in"] = nc.dram_tensor("kv_cc_in", [2, KVLEN], F32)
    T["kv_cc_out"] = nc.dram_tensor("kv_cc_out", [2, KVLEN], F32)
    T["ch_cc_in"] = nc.dram_tensor("ch_cc_in", [2, 64 * HEADS * 64], F32)
    T["ch_cc_out"] = nc.dram_tensor("ch_cc_out", [2, 64 * HEADS * 64], F32)

    import contextlib
    with tile.TileContext(nc) as tc:
        with contextlib.ExitStack() as ctx:
            _build_body(ctx, nc, tc, groups, T)
    nc.compile()
    return nc


def _build_body(ctx, nc, tc, groups, T):
    W = T["W"]
    KVLEN = T["KVLEN"]
    DBG = os.environ.get("KB_DEBUG", "")
    dbg_written = [False]

    def dbg_dump(nc_, name, tiles):
        if name != DBG or dbg_written[0]:
            return
        dbg_written[0] = True
        off = 0
        for t, ncols in tiles:
            nparts = t.shape[0]
            nc_.sync.dma_start(
                out=T["dbg_out"][0:nparts, off:off + ncols], in_=t)
            off += ncols

    def pool(name, bufs, space="SBUF"):
        return ctx.enter_context(tc.tile_pool(name=name, bufs=bufs, space=space))

    const = pool("const", 1)
    persist = pool("persist", 2)
    xpool = pool("xpool", 5)
    xnpool = pool("xnpool", 4)
    ctxp = pool("ctxp", 2)
    zp = pool("zp", 4)
    xtp = pool("xtp", 6)
    xtrev = pool("xtrev", 6)
    qkp = pool("qkp", 5)
    vkm = pool("vkm", 3)
    vtp = pool("vtp", 6)
    kvp = pool("kvp", 1)
    vap = pool("vap", 1)
    ep = pool("ep", 5)
    bp = pool("bp", 2)
    otp = pool("otp", 8)
    onp = pool("onp", 2)
    wpool = pool("wpool", 4)
    gp = pool("gp", 2)
    spool = pool("spool", 3)
    denp = pool("denp", 2)
    chqk = pool("chqk", 2)
    chvp = pool("chvp", 3)
    chs = pool("chs", 2)
    ffp = pool("ffp", 8)
    ps_mm = pool("ps_mm", 2, "PSUM")
    ps_tr2 = pool("ps_tr2", 2, "PSUM")
    ps_av = pool("ps_av", 4, "PSUM")

    def bcast_vec(dst_pool, dram, dlen, tag, parts=P):
        t = dst_pool.tile([parts, dlen], F32, tag=tag)
        a = dram[:]
        nc.sync.dma_start(out=t, in_=bass.AP(tensor=a.tensor, offset=0,
                                             ap=[[0, parts], [1, dlen]]))
        return t

    rev_t = const.tile([P, P], F32)
    nc.sync.dma_start(out=rev_t, in_=T["rev_in"][:])
    ident_t = const.tile([P, P], F32)
    nc.sync.dma_start(out=ident_t, in_=T["ident_in"][:])
    ma = T["mask_in"][:]
    m0 = const.tile([P, 1], F32)
    m1 = const.tile([P, 1], F32)
    nc.sync.dma_start(out=m0, in_=bass.AP(tensor=ma.tensor, offset=0, ap=[[0, P], [1, 1]]))
    nc.sync.dma_start(out=m1, in_=bass.AP(tensor=ma.tensor, offset=1, ap=[[0, P], [1, 1]]))
    cosq = const.tile([P, R], F32)
    nc.sync.dma_start(out=cosq, in_=T["cosq_in"][:])
    sinq = const.tile([P, R], F32)
    nc.sync.dma_start(out=sinq, in_=T["sinq_in"][:])
    cosk = const.tile([DIM_HEAD, R], F32)
    nc.sync.dma_start(out=cosk, in_=T["cosk_in"][:])
    sink = const.tile([DIM_HEAD, R], F32)
    nc.sync.dma_start(out=sink, in_=T["sink_in"][:])
    magic = const.tile([P, 16], I32)
    nc.vector.memset(magic, 0x5f3759df)
    ones_t = const.tile([P, P], F32)
    nc.vector.memset(ones_t, 1.0)

    biasap = T["bias_in"][:]

    def bias_src(h, kind, off, n):
        return bass.AP(tensor=biasap.tensor, offset=h * 4096 + kind * 2048 + off,
                       ap=[[1, P], [1, n]])

    def copy_any(dst, src, use_act=True):
        if use_act:
            nc.scalar.copy(out=dst, in_=src)
        else:
            nc.vector.tensor_copy(out=dst, in_=src)

    def evict(dst, src_psum, eng=None):
        if eng is nc.vector:
            nc.vector.tensor_copy(out=dst, in_=src_psum)
        else:
            nc.scalar.copy(out=dst, in_=src_psum)

    def rsqrt_newton(v):
        n = v.shape[1]
        r = spool.tile([P, n], F32, tag="nrt_r")
        iv = r.bitcast(I32)
        nc.vector.tensor_scalar(out=iv, in0=v.bitcast(I32), scalar1=1,
                                scalar2=None, op0=ALU.arith_shift_right)
        nc.vector.tensor_tensor(out=iv, in0=magic[:, 0:n], in1=iv, op=ALU.subtract)
        t = spool.tile([P, n], F32, tag="nrt_t")
        for _ in range(3):
            nc.vector.tensor_tensor(out=t, in0=r, in1=r, op=ALU.mult)
            nc.vector.tensor_tensor(out=t, in0=t, in1=v, op=ALU.mult)
            nc.vector.tensor_scalar(out=t, in0=t, scalar1=-0.5, scalar2=1.5,
                                    op0=ALU.mult, op1=ALU.add)
            nc.vector.tensor_tensor(out=r, in0=r, in1=t, op=ALU.mult)
        nc.vector.tensor_copy(out=v, in_=r)
        return v

    def layernorm(tiles, d, out_pool, tag):
        nt = len(tiles)
        sub = math.gcd(512, d)
        nsub = d // sub
        var = spool.tile([P, nt], F32, tag="var")
        mean = spool.tile([P, nt], F32, tag="mean")
        for i, t in enumerate(tiles):
            stats = spool.tile([P, nsub, 6], F32, tag="bnstats")
            src3 = t.rearrange("p (s q) -> p s q", s=nsub)
            for s in range(nsub):
                nc.vector.bn_stats(out=stats[:, s, :], in_=src3[:, s, :])
            mv = spool.tile([P, 2], F32, tag="mv")
            nc.vector.bn_aggr(out=mv, in_=stats)
            nc.vector.tensor_copy(out=mean[:, i:i + 1], in_=mv[:, 0:1])
            nc.vector.tensor_copy(out=var[:, i:i + 1], in_=mv[:, 1:2])
        nc.vector.tensor_scalar_add(out=var, in0=var, scalar1=EPS)
        rstd = rsqrt_newton(var)
        nmr = spool.tile([P, nt], F32, tag="nmr")
        nc.vector.tensor_tensor(out=nmr, in0=mean, in1=rstd, op=ALU.mult)
        nc.vector.tensor_scalar_mul(out=nmr, in0=nmr, scalar1=-1.0)
        outs = []
        for i, t in enumerate(tiles):
            o = t if out_pool is None else out_pool.tile([P, d], F32, tag=tag)
            nc.scalar.activation(out=o, in_=t, func=AF.Identity,
                                 bias=nmr[:, i:i + 1], scale=rstd[:, i:i + 1])
            outs.append(o)
        return outs

    def transpose_to_feature(tiles, d, tag, reverse=False):
        nt = len(tiles)
        nkt = d // P
        dst_pool = xtrev if reverse else xtp
        perm = rev_t if reverse else ident_t
        outs = []
        for kt in range(nkt):
            ot = dst_pool.tile([P, nt * P], F32, tag=tag)
            for rt in range(nt):
                src = tiles[nt - 1 - rt] if reverse else tiles[rt]
                pst = ps_mm.tile([P, P], F32, tag="mm")
                nc.tensor.transpose(pst, src[:, kt * P:(kt + 1) * P], perm)
                copy_any(ot[:, rt * P:(rt + 1) * P], pst, use_act=(rt + kt) % 2 == 0)
            outs.append(ot)
        return outs

    def wslice(wdram, kt, c0, c1):
        t = wpool.tile([P, c1 - c0], MMDT, tag="w")
        nc.sync.dma_start(out=t, in_=wdram[kt * P:(kt + 1) * P, c0:c1])
        return t

    def feat_mm(psum, wdram, nkt, c0, c1, rhs_tiles):
        for i in range(nkt):
            nc.tensor.matmul(psum, wslice(wdram, i, c0, c1), rhs_tiles[i],
                             start=(i == 0), stop=(i == nkt - 1))

    din0 = CFGS[0]["self_in"]
    x_tiles = []
    for rt in range(4):
        xt0 = xpool.tile([P, din0], F32, tag="x")
        nc.sync.dma_start(out=xt0, in_=T["x_in"][rt * P:(rt + 1) * P, :])
        x_tiles.append(xt0)

    ctxnT = []
    for kt in range(2):
        ot = persist.tile([P, 8 * P], F32, tag="ctxnT")
        ctxnT.append(ot)
    for c0 in range(0, 8, 2):
        pairtiles = []
        for ct in (c0, c0 + 1):
            ctt = ctxp.tile([P, PFD], F32, tag="ctxload")
            nc.sync.dma_start(out=ctt, in_=T["ctx_in"][ct * P:(ct + 1) * P, :])
            pairtiles.append(ctt)
        pairn = layernorm(pairtiles, PFD, None, None)
        for j, ct in enumerate((c0, c0 + 1)):
            for kt in range(2):
                pst = ps_mm.tile([P, P], F32, tag="mm")
                nc.tensor.transpose(pst, pairn[j][:, kt * P:(kt + 1) * P], ident_t)
                copy_any(ctxnT[kt][:, ct * P:(ct + 1) * P], pst,
                         use_act=(ct + kt) % 2 == 0)

    def attention_core(qts, kT2, vaug, with_bias):
        outT = []
        for grp in range(2):
            av = []
            for _avi in range(4):
                avt = ps_av.tile([DIM_HEAD + 1, R], F32, tag="av")
                av.append(avt)
            for jt in range(JT):
                for pairi in range(2):
                    qtile = qts[grp * 2 + pairi]
                    for sub in range(2):
                        h = grp * 4 + pairi * 2 + sub
                        pss = ps_mm.tile([P, R], F32, tag="mm")
                        nc.tensor.matmul(
                            pss,
                            kT2[sub * 64:sub * 64 + 64, jt * P:(jt + 1) * P],
                            qtile[sub * 64:sub * 64 + 64, :],
                            start=True, stop=True,
                            tile_position=(sub * 64, 0),
                        )
                        e = ep.tile([P, R], F32, tag="E")
                        nc.scalar.activation(out=e, in_=pss, func=AF.Exp)
                        if with_bias:
                            bt = bp.tile([P, R], F32, tag="bias")
                            if jt < 4:
                                bsrc = bias_src(h, 0, jt * P + 512, R)
                            elif jt < 8:
                                bsrc = bias_src(h, 0, jt * P - 512, R)
                            else:
                                bsrc = bias_src(h, 1, 0, R)
                            nc.sync.dma_start(out=bt, in_=bsrc)
                            nc.vector.tensor_tensor(out=e, in0=e, in1=bt, op=ALU.mult)
                        nc.tensor.matmul(av[h - grp * 4], vaug[:, jt, :], e,
                                         start=(jt == 0), stop=(jt == JT - 1))
            for ai in range(4):
                a0 = av[ai]
                den = denp.tile([P, R], F32, tag="den")
                nc.scalar.copy(out=den[64:65, :], in_=a0[64:65, :])
                nc.vector.reciprocal(out=den[64:65, :], in_=den[64:65, :])
                psb = ps_mm.tile([P, R], F32, tag="mm")
                nc.tensor.matmul(psb, ones_t[64:65, :], den[64:65, :],
                                 start=True, stop=True, tile_position=(64, 0))
                onum = onp.tile([DIM_HEAD, R], F32, tag="onum")
                nc.scalar.copy(out=onum, in_=a0[0:64, :])
                ot = otp.tile([DIM_HEAD, R], F32, tag="outT")
                nc.vector.tensor_tensor(out=ot, in0=onum, in1=psb[0:64, :],
                                        op=ALU.mult)
                outT.append(ot)
        return outT

    def wo_block(outT, wodram, dout, gdram, x_tiles_in, residual):
        gt = bcast_vec(gp, gdram, dout, "gout")
        new_x = []
        for rt in range(4):
            zrow = zp.tile([P, dout], F32, tag="zrow")
            for ch in range((dout + 511) // 512):
                c0, c1 = ch * 512, min(dout, ch * 512 + 512)
                psy = ps_mm.tile([P, c1 - c0], F32, tag="mm")
                for h in range(HEADS):
                    wt = wpool.tile([DIM_HEAD, c1 - c0], MMDT, tag="w")
                    nc.sync.dma_start(out=wt, in_=wodram[h * 64:(h + 1) * 64, c0:c1])
                    nc.tensor.matmul(psy, outT[h][:, rt * P:(rt + 1) * P], wt,
                                     start=(h == 0), stop=(h == HEADS - 1))
                evict(zrow[:, c0:c1], psy)
            zl = layernorm([zrow], dout, None, None)[0]
            nc.vector.tensor_tensor(out=zl, in0=zl, in1=gt, op=ALU.mult)
            nx = xpool.tile([P, dout], F32, tag="x")
            if residual:
                nc.vector.tensor_tensor(out=nx, in0=zl, in1=x_tiles_in[rt], op=ALU.add)
            else:
                nc.vector.tensor_copy(out=nx, in_=zl)
            new_x.append(nx)
        return new_x

    for li, c in enumerate(CFGS):
        lw = W[li]
        din, d, dco, dff = c["self_in"], c["d"], c["chan_out"], c["ff_d"]
        inner = FF_MULT * dff
        nkt = din // P

        xn = layernorm(x_tiles, din, xnpool, "xn")
        if li == 0:
            dbg_dump(nc, "xn0", [(t, din) for t in xn])
        xnT = transpose_to_feature(xn, din, "xnT")
        xnTrev = transpose_to_feature(xn, din, "xnTrev", reverse=True)
        if li == 0:
            dbg_dump(nc, "xnT0", [(t, 512) for t in xnT])
            dbg_dump(nc, "xnTrev0", [(t, 512) for t in xnTrev])

        qts = []
        for qt in range(4):
            psq = ps_mm.tile([P, R], F32, tag="mm")
            feat_mm(psq, lw["wq"], nkt, qt * P, qt * P + P, xnT)
            psqs = ps_tr2.tile([P, R], F32, tag="trq")
            feat_mm(psqs, lw["wqs"], nkt, qt * P, qt * P + P, xnT)
            t1 = qkp.tile([P, R], F32, tag="qrot")
            nc.vector.tensor_tensor(out=t1, in0=psq, in1=cosq, op=ALU.mult)
            t2 = qkp.tile([P, R], F32, tag="qrot")
            nc.vector.tensor_tensor(out=t2, in0=psqs, in1=sinq, op=ALU.mult)
            nc.vector.tensor_tensor(out=t1, in0=t1, in1=t2, op=ALU.add)
            qts.append(t1)
        if li == 0:
            dbg_dump(nc, "q0", [(t, R) for t in qts])

        psk = ps_mm.tile([DIM_HEAD, R], F32, tag="mm")
        feat_mm(psk, lw["wk"], nkt, 0, DIM_HEAD, xnTrev)
        psks = ps_tr2.tile([DIM_HEAD, R], F32, tag="trq")
        feat_mm(psks, lw["wks"], nkt, 0, DIM_HEAD, xnTrev)
        krot = vkm.tile([DIM_HEAD, R], F32, tag="kmask")
        nc.vector.tensor_tensor(out=krot, in0=psk, in1=cosk, op=ALU.mult)
        ktmp = vkm.tile([DIM_HEAD, R], F32, tag="kmask")
        nc.vector.tensor_tensor(out=ktmp, in0=psks, in1=sink, op=ALU.mult)
        nc.vector.tensor_tensor(out=krot, in0=krot, in1=ktmp, op=ALU.add)

        v_tiles = []
        for vt in range(4):
            psv = ps_mm.tile([P, DIM_HEAD], F32, tag="mm")
            for i in range(nkt):
                wt = wslice(lw["wv"], i, 0, DIM_HEAD)
                nc.tensor.matmul(psv, xnTrev[i][:, vt * P:(vt + 1) * P], wt,
                                 start=(i == 0), stop=(i == nkt - 1))
            sv = vtp.tile([P, DIM_HEAD], F32, tag="vtile")
            evict(sv, psv)
            v_tiles.append(sv)

        for slot, msk in ((0, m0), (1, m1)):
            km = vkm.tile([DIM_HEAD, R], F32, tag="kmask")
            nc.vector.tensor_scalar_mul(out=km, in0=krot, scalar1=msk[0:DIM_HEAD])
            nc.sync.dma_start(
                out=T["kv_cc_in"][slot, 0:64 * R].rearrange("(p f) -> p f", p=64),
                in_=km)
            for vt in range(4):
                vm = vtp.tile([P, DIM_HEAD], F32, tag="vtile")
                nc.vector.tensor_scalar_mul(out=vm, in0=v_tiles[vt], scalar1=msk)
                nc.sync.dma_start(
                    out=T["kv_cc_in"][slot, 64 * R + vt * P * 64:
                                      64 * R + (vt + 1) * P * 64]
                    .rearrange("(p f) -> p f", p=P),
                    in_=vm)
        nc.gpsimd.collective_compute(
            "AllReduce", ALU.add, replica_groups=groups,
            ins=[T["kv_cc_in"][:]], outs=[T["kv_cc_out"][:]])

        kT2 = kvp.tile([P, JT * P], F32, tag="kT2")
        nc.vector.memset(kT2[:, 8 * P:JT * P], 0.0)
        for half in range(2):
            for slot in range(2):
                nc.sync.dma_start(
                    out=kT2[half * 64:half * 64 + 64, slot * R:(slot + 1) * R],
                    in_=T["kv_cc_out"][slot, 0:64 * R].rearrange("(p f) -> p f", p=64))
            nka = lw["null_k"][:]
            nc.sync.dma_start(
                out=kT2[half * 64:half * 64 + 64, 8 * P:8 * P + 1],
                in_=bass.AP(tensor=nka.tensor, offset=0, ap=[[1, 64], [1, 1]]))
        vaug = vap.tile([P, JT, DIM_HEAD + 1], F32, tag="vaug")
        nc.vector.memset(vaug[:, 8, :], 0.0)
        nc.vector.memset(vaug[:, 0:8, 64:65], 1.0)
        for slot in range(2):
            nc.sync.dma_start(
                out=vaug[:, slot * 4:slot * 4 + 4, 0:64],
                in_=T["kv_cc_out"][slot, 64 * R:KVLEN]
                .rearrange("(t p f) -> p t f", p=P, t=4))
        nva = lw["null_v"][:]
        nc.sync.dma_start(out=vaug[0:1, 8, 0:64],
                          in_=bass.AP(tensor=nva.tensor, offset=0, ap=[[0, 1], [1, 64]]))
        nc.vector.memset(vaug[0:1, 8, 64:65], 1.0)
        if li == 0:
            dbg_dump(nc, "kT2", [(kT2, JT * P)])
            dbg_dump(nc, "vaug", [(vaug.rearrange("p t f -> p (t f)"), JT * 65)])
            dbg_dump(nc, "krot", [(krot, R)])

        outT = attention_core(qts, kT2, vaug, with_bias=True)
        if li == 0:
            dbg_dump(nc, "sa_outT", [(t, R) for t in outT])
        x_tiles = wo_block(outT, lw["wo_self"], d, lw["g_self_out"], x_tiles,
                           residual=(li != 0))
        if li == 0:
            dbg_dump(nc, "x_sa", [(t, d) for t in x_tiles])

        xn = layernorm(x_tiles, d, xnpool, "xn")
        xnT = transpose_to_feature(xn, d, "xnT")
        ndkt = d // P
        qts = []
        for qt in range(4):
            psq = ps_mm.tile([P, R], F32, tag="mm")
            feat_mm(psq, lw["xwq"], ndkt, qt * P, qt * P + P, xnT)
            t1 = qkp.tile([P, R], F32, tag="qrot")
            evict(t1, psq)
            qts.append(t1)
        kT2x = kvp.tile([P, JT * P], F32, tag="kT2")
        nc.vector.memset(kT2x[:, 8 * P:JT * P], 0.0)
        for nch in range(2):
            pskx = ps_mm.tile([DIM_HEAD, 512], F32, tag="mm")
            for i in range(2):
                wt = wslice(lw["xwk"], i, 0, DIM_HEAD)
                nc.tensor.matmul(pskx, wt, ctxnT[i][:, nch * 512:(nch + 1) * 512],
                                 start=(i == 0), stop=(i == 1))
            evict(kT2x[0:64, nch * 512:(nch + 1) * 512], pskx)
        nka = lw["xnull_k"][:]
        nc.sync.dma_start(
            out=kT2x[0:64, 8 * P:8 * P + 1],
            in_=bass.AP(tensor=nka.tensor, offset=0, ap=[[1, 64], [1, 1]]))
        nc.sync.dma_start(out=kT2x[64:128, :], in_=kT2x[0:64, :])
        vaugx = vap.tile([P, JT, DIM_HEAD + 1], F32, tag="vaug")
        nc.vector.memset(vaugx[:, 8, :], 0.0)
        nc.vector.memset(vaugx[:, 0:8, 64:65], 1.0)
        for jt in range(8):
            psvx = ps_mm.tile([P, DIM_HEAD], F32, tag="mm")
            for i in range(2):
                wt = wslice(lw["xwv"], i, 0, DIM_HEAD)
                nc.tensor.matmul(psvx, ctxnT[i][:, jt * P:(jt + 1) * P], wt,
                                 start=(i == 0), stop=(i == 1))
            evict(vaugx[:, jt, 0:64], psvx, eng=nc.vector)
        nva = lw["xnull_v"][:]
        nc.sync.dma_start(out=vaugx[0:1, 8, 0:64],
                          in_=bass.AP(tensor=nva.tensor, offset=0, ap=[[0, 1], [1, 64]]))
        nc.vector.memset(vaugx[0:1, 8, 64:65], 1.0)

        outT = attention_core(qts, kT2x, vaugx, with_bias=False)
        x_tiles = wo_block(outT, lw["wo_cross"], d, lw["g_cross_out"], x_tiles,
                           residual=True)
        if li == 0:
            dbg_dump(nc, "x_cross", [(t, d) for t in x_tiles])

        xn = layernorm(x_tiles, d, xnpool, "xn")
        xnT = transpose_to_feature(xn, d, "xnT")
        sims = chs.tile([64, HEADS, 64], F32, tag="chsim")
        for rt in range(4):
            psq = ps_mm.tile([P, INNER], F32, tag="mm")
            for i in range(ndkt):
                wt = wslice(lw["cwqkv"], i, 0, INNER)
                nc.tensor.matmul(psq, xnT[i][:, rt * P:(rt + 1) * P], wt,
                                 start=(i == 0), stop=(i == ndkt - 1))
            tq = chqk.tile([P, INNER], F32, tag="chq")
            evict(tq, psq)
            psk2 = ps_mm.tile([P, INNER], F32, tag="mm")
            for i in range(ndkt):
                wt = wslice(lw["cwqkv"], i, INNER, 2 * INNER)
                nc.tensor.matmul(psk2, xnT[i][:, rt * P:(rt + 1) * P], wt,
                                 start=(i == 0), stop=(i == ndkt - 1))
            tk = chqk.tile([P, INNER], F32, tag="chk")
            evict(tk, psk2, eng=nc.vector)
            for h in range(HEADS):
                pss = ps_mm.tile([64, 64], F32, tag="mm")
                nc.tensor.matmul(pss, tq[:, h * 64:(h + 1) * 64],
                                 tk[:, h * 64:(h + 1) * 64],
                                 start=True, stop=True)
                if rt == 0:
                    evict(sims[:, h, :], pss, eng=nc.vector if h % 2 else None)
                else:
                    nc.vector.tensor_tensor(out=sims[:, h, :], in0=sims[:, h, :],
                                            in1=pss, op=ALU.add)
        for slot, msk in ((0, m0), (1, m1)):
            sm = chs.tile([64, HEADS, 64], F32, tag="chsimm")
            nc.vector.tensor_scalar_mul(out=sm, in0=sims, scalar1=msk[0:64])
            nc.sync.dma_start(
                out=T["ch_cc_in"][slot].rearrange("(p f) -> p f", p=64),
                in_=sm.rearrange("p h f -> p (h f)"))
        nc.gpsimd.collective_compute(
            "AllReduce", ALU.add, replica_groups=groups,
            ins=[T["ch_cc_in"][:]], outs=[T["ch_cc_out"][:]])
        simf = chs.tile([64, HEADS, 64], F32, tag="chsim")
        nc.sync.dma_start(
            out=simf.rearrange("p h f -> p (h f)"),
            in_=T["ch_cc_out"][0].rearrange("(p f) -> p f", p=64))
        sim2 = chs.tile([64, HEADS, 64], F32, tag="chsimm")
        nc.sync.dma_start(
            out=sim2.rearrange("p h f -> p (h f)"),
            in_=T["ch_cc_out"][1].rearrange("(p f) -> p f", p=64))
        nc.vector.tensor_tensor(out=simf, in0=simf, in1=sim2, op=ALU.add)
        mx = spool.tile([64, HEADS], F32, tag="chmx")
        nc.vector.tensor_reduce(out=mx, in_=simf, axis=mybir.AxisListType.X,
                                op=ALU.max, negate=True)
        att = chs.tile([64, HEADS, 64], F32, tag="chsimm")
        for h in range(HEADS):
            nc.scalar.activation(out=att[:, h, :], in_=simf[:, h, :], func=AF.Exp,
                                 bias=mx[:, h:h + 1], scale=1.0)
        ssum = spool.tile([64, HEADS], F32, tag="chsum")
        nc.vector.tensor_reduce(out=ssum, in_=att, axis=mybir.AxisListType.X,
                                op=ALU.add)
        nc.vector.reciprocal(out=ssum, in_=ssum)
        for h in range(HEADS):
            nc.vector.tensor_scalar_mul(out=att[:, h, :], in0=att[:, h, :],
                                        scalar1=ssum[:, h:h + 1])
        outT = []
        for h in range(HEADS):
            psv = ps_mm.tile([DIM_HEAD, R], F32, tag="mm")
            feat_mm(psv, lw["cwqkv"], ndkt, 2 * INNER + h * 64,
                    2 * INNER + (h + 1) * 64, xnT)
            tv = chvp.tile([DIM_HEAD, R], F32, tag="chv")
            evict(tv, psv)
            psT = ps_mm.tile([64, 64], F32, tag="mm")
            nc.tensor.transpose(psT, att[:, h, :], ident_t[0:64, 0:64])
            attT = spool.tile([64, 64], F32, tag="chattT")
            evict(attT, psT)
            pso = ps_mm.tile([64, R], F32, tag="mm")
            nc.tensor.matmul(pso, attT, tv, start=True, stop=True)
            ot = otp.tile([DIM_HEAD, R], F32, tag="outT")
            evict(ot, pso, eng=None if h % 2 else nc.vector)
            outT.append(ot)
        x_tiles = wo_block(outT, lw["wo_chan"], dco, lw["g_chan_out"], x_tiles,
                           residual=(li != DEPTH + 1))
        if li == 0:
            dbg_dump(nc, "x_chan", [(t, dco) for t in x_tiles])

        xn = layernorm(x_tiles, dff, xnpool, "xn")
        xnT = transpose_to_feature(xn, dff, "xnT")
        nffk = dff // P
        nI = inner // P
        HPASS = 6
        ysum = [None] * 4
        for base in range(0, nI, HPASS):
            cnt = min(HPASS, nI - base)
            gact = []
            for ii in range(cnt):
                i = base + ii
                psa = ps_mm.tile([P, R], F32, tag="mm")
                feat_mm(psa, lw["w1"], nffk, i * P, (i + 1) * P, xnT)
                psg = ps_tr2.tile([P, R], F32, tag="trq")
                feat_mm(psg, lw["w1"], nffk, inner + i * P, inner + (i + 1) * P, xnT)
                tg = ffp.tile([P, R], F32, tag="gact")
                th = ffp.tile([P, R], F32, tag="gact")
                nc.scalar.activation(out=th, in_=psg, func=AF.Tanh, scale=0.5)
                nc.vector.tensor_scalar(out=th, in0=th, scalar1=0.5, scalar2=0.5,
                                        op0=ALU.mult, op1=ALU.add)
                nc.vector.tensor_tensor(out=th, in0=th, in1=psg, op=ALU.mult)
                nc.vector.tensor_tensor(out=tg, in0=th, in1=psa, op=ALU.mult)
                gact.append(tg)
            for rt in range(4):
                for ch in range((dff + 511) // 512):
                    c0, c1 = ch * 512, min(dff, ch * 512 + 512)
                    psy = ps_mm.tile([P, c1 - c0], F32, tag="mm")
                    for ii in range(cnt):
                        i = base + ii
                        wt = wslice(lw["w2"], i, c0, c1)
                        nc.tensor.matmul(psy, gact[ii][:, rt * P:(rt + 1) * P], wt,
                                         start=(ii == 0), stop=(ii == cnt - 1))
                    if base == 0:
                        if ysum[rt] is None:
                            zr = zp.tile([P, dff], F32, tag="zrow")
                            ysum[rt] = zr
                        evict(ysum[rt][:, c0:c1], psy)
                    else:
                        nc.vector.tensor_tensor(out=ysum[rt][:, c0:c1],
                                                in0=ysum[rt][:, c0:c1], in1=psy,
                                                op=ALU.add)
        new_x = []
        for rt in range(4):
            nx = xpool.tile([P, dff], F32, tag="x")
            nc.vector.tensor_tensor(out=nx, in0=ysum[rt], in1=x_tiles[rt], op=ALU.add)
            new_x.append(nx)
        x_tiles = new_x

    gfin = bcast_vec(gp, T["gfin_in"], DLAST, "gout")
    xdiv = []
    for rt in range(4):
        mxv = spool.tile([P, 1], F32, tag="fmax")
        nc.vector.tensor_reduce(out=mxv, in_=x_tiles[rt], axis=mybir.AxisListType.X,
                                op=ALU.max)
        nc.vector.reciprocal(out=mxv, in_=mxv)
        td = xnpool.tile([P, DLAST], F32, tag="xn")
        nc.vector.tensor_scalar_mul(out=td, in0=x_tiles[rt], scalar1=mxv)
        xdiv.append(td)
    xfin = layernorm(xdiv, DLAST, None, None)
    for rt in range(4):
        nc.vector.tensor_tensor(out=xfin[rt], in0=xfin[rt], in1=gfin, op=ALU.mult)
        nc.sync.dma_start(out=T["y_out"][rt * P:(rt + 1) * P, :], in_=xfin[rt])



_CACHE = {}


def _get_nc():
    key = (NLAYERS, str(MMDT), os.environ.get("KB_DEBUG", ""))
    if key not in _CACHE:
        _CACHE[key] = build_program()
    return _CACHE[key]


def make_in_maps(x, context, params):
    x = np.asarray(x, np.float32)
    context = np.asarray(context, np.float32)
    folded = fold_weights(params)
    emb = np.asarray(params["rel_pos_emb"], np.float32)
    gfin = np.asarray(params["out_norm_g"], np.float32)[:DLAST]

    rev = np.zeros((P, P), np.float32)
    rev[np.arange(P), P - 1 - np.arange(P)] = 1.0
    ident = np.eye(P, dtype=np.float32)

    in_maps = []
    for core in range(NCORES):
        b = core // 2
        parity = core % 2
        cosq, sinq, cosk, sink = make_rotary(parity)
        im = {
            "x_in": np.ascontiguousarray(x[b, parity * R:(parity + 1) * R, :]),
            "ctx_in": np.ascontiguousarray(context[b]),
            "rev_in": rev, "ident_in": ident,
            "mask_in": np.array([1.0 - parity, float(parity)], np.float32),
            "cosq_in": cosq, "sinq_in": sinq, "cosk_in": cosk, "sink_in": sink,
            "bias_in": make_bias_tables(emb, parity),
            "gfin_in": gfin,
        }
        for li, L in enumerate(folded):
            for nm, v in L.items():
                im[f"L{li}_{nm}"] = np.ascontiguousarray(v)
        in_maps.append(im)
    return in_maps


def kernel(x, context, params):
    nc = _get_nc()
    in_maps = make_in_maps(x, context, params)
    res = run_bass_kernel_spmd(nc, in_maps, core_ids=list(range(NCORES)))
    out = np.zeros((B, N, DLAST), np.float32)
    for core in range(NCORES):
        b, parity = core // 2, core % 2
        out[b, parity * R:(parity + 1) * R, :] = res.results[core]["y_out"]
    return out


# revision 14
# speedup vs baseline: 1.0330x; 1.0330x over previous
import sys
import os

sys.path.insert(0, "/opt/trn_rl_repo")

import numpy as np
import math

import concourse.bass as bass
import concourse.tile as tile
from concourse import bacc, mybir
from concourse.bass_utils import run_bass_kernel_spmd

F32 = mybir.dt.float32
I32 = mybir.dt.int32
AF = mybir.ActivationFunctionType
ALU = mybir.AluOpType

DIM = 512
IN_OUT = 768
DEPTH = 4
HEADS = 8
DIM_HEAD = 64
INNER = 512
PFD = 256
FF_MULT = 4
ROT_DIM = 32
NUM_BUCKETS = 32
MAX_DIST = 128
B, N, M = 4, 1024, 1024
NLAYERS = int(os.environ.get("KB_NLAYERS", str(DEPTH + 2)))
R = 512
P = 128
NCORES = 8
JT = 9
EPS = 1e-5
MMDT = getattr(mybir.dt, os.environ.get("KB_MMDT", "float32"))


def _layer_cfgs():
    cfgs = [dict(self_in=IN_OUT, d=DIM, chan_out=DIM, ff_d=DIM)]
    for _ in range(DEPTH):
        cfgs.append(dict(self_in=DIM, d=DIM, chan_out=DIM, ff_d=DIM))
    cfgs.append(dict(self_in=DIM, d=DIM, chan_out=IN_OUT, ff_d=IN_OUT))
    return cfgs


CFGS = _layer_cfgs()[:NLAYERS]
DLAST = IN_OUT if NLAYERS == DEPTH + 2 else CFGS[-1]["d"]



def _bucket_table():
    neg = np.arange(0, 2050)
    max_exact = NUM_BUCKETS // 2
    is_small = neg < max_exact
    large = max_exact + (
        np.log(np.maximum(neg, 1) / max_exact)
        / math.log(MAX_DIST / max_exact)
        * (NUM_BUCKETS - max_exact)
    ).astype(np.int64)
    large = np.minimum(large, NUM_BUCKETS - 1)
    return np.where(is_small, neg, large)


def make_bias_tables(emb, parity):
    bt = _bucket_table()
    emb = np.asarray(emb, np.float64)
    tabs = np.zeros((HEADS, 2, 2048), np.float32)
    s = np.arange(2048)
    d = s - 1023 + 512 * parity
    valid = (d >= 0) & (d < N)
    negidx = np.clip(np.maximum(d - 1, 0), 0, 2049)
    u = np.arange(2048) + 512 * parity
    uvalid = u < N
    for h in range(HEADS):
        vals = np.exp(emb[bt[negidx], h])
        tabs[h, 0, :] = np.where(valid, vals, 0.0)
        tabs[h, 1, :] = np.where(uvalid, np.exp(emb[bt[np.clip(u, 0, 2049)], h]), 0.0)
    return tabs


def make_rotary(parity):
    inv_freq = 1.0 / (10000.0 ** (np.arange(0, ROT_DIM, 2, dtype=np.float32) / ROT_DIM))
    pos_all = np.arange(N, dtype=np.float32)[:, None] * inv_freq[None, :].astype(np.float32)
    pos_all = np.concatenate([pos_all, pos_all], axis=1)
    rows = np.arange(R) + R * parity
    rows_rev = (R - 1 - np.arange(R)) + R * parity
    cosq = np.ones((P, R), np.float32)
    sinq = np.zeros((P, R), np.float32)
    for p in range(P):
        dd = p % DIM_HEAD
        if dd < ROT_DIM:
            cosq[p, :] = np.cos(pos_all[rows, dd])
            sinq[p, :] = np.sin(pos_all[rows, dd])
    cosk = np.ones((DIM_HEAD, R), np.float32)
    sink = np.zeros((DIM_HEAD, R), np.float32)
    for dd in range(DIM_HEAD):
        if dd < ROT_DIM:
            cosk[dd, :] = np.cos(pos_all[rows_rev, dd])
            sink[dd, :] = np.sin(pos_all[rows_rev, dd])
    return cosq, sinq, cosk, sink


def _sigma(w):
    ws = np.zeros_like(w)
    nh = w.shape[1] // DIM_HEAD
    for h in range(nh):
        b = h * DIM_HEAD
        ws[:, b:b + 16] = -w[:, b + 16:b + 32]
        ws[:, b + 16:b + 32] = w[:, b:b + 16]
    return ws


def fold_weights(params):
    scale = DIM_HEAD ** -0.5
    out = []
    for lp in params["layers"][:NLAYERS]:
        L = {}
        sp = lp["self"]
        g = np.asarray(sp["norm_g"], np.float32)
        wq = np.asarray(sp["wq"], np.float32) * g[:, None] * scale
        wkv = np.asarray(sp["wkv"], np.float32) * g[:, None]
        L["wq"] = wq
        L["wqs"] = _sigma(wq)
        L["wk"] = np.ascontiguousarray(wkv[:, :DIM_HEAD])
        L["wks"] = _sigma(L["wk"])
        L["wv"] = np.ascontiguousarray(wkv[:, DIM_HEAD:])
        L["null_k"] = np.ascontiguousarray(np.asarray(sp["null_kv"], np.float32)[0])
        L["null_v"] = np.ascontiguousarray(np.asarray(sp["null_kv"], np.float32)[1])
        L["wo_self"] = np.asarray(sp["wo"], np.float32)
        L["g_self_out"] = np.asarray(sp["out_norm_g"], np.float32)
        cp = lp["cross"]
        g = np.asarray(cp["norm_g"], np.float32)
        L["xwq"] = np.asarray(cp["wq"], np.float32) * g[:, None] * scale
        gc = np.asarray(cp["ctx_norm_g"], np.float32)
        xwkv = np.asarray(cp["wkv"], np.float32) * gc[:, None]
        L["xwk"] = np.ascontiguousarray(xwkv[:, :DIM_HEAD])
        L["xwv"] = np.ascontiguousarray(xwkv[:, DIM_HEAD:])
        L["xnull_k"] = np.ascontiguousarray(np.asarray(cp["null_kv"], np.float32)[0])
        L["xnull_v"] = np.ascontiguousarray(np.asarray(cp["null_kv"], np.float32)[1])
        L["wo_cross"] = np.asarray(cp["wo"], np.float32)
        L["g_cross_out"] = np.asarray(cp["out_norm_g"], np.float32)
        hp = lp["chan"]
        g = np.asarray(hp["norm_g"], np.float32)
        wqkv = (np.asarray(hp["wqkv"], np.float32) * g[:, None]).copy()
        wqkv[:, :INNER] *= scale
        L["cwqkv"] = wqkv
        L["wo_chan"] = np.asarray(hp["wo"], np.float32)
        L["g_chan_out"] = np.asarray(hp["out_norm_g"], np.float32)
        fp = lp["ff"]
        g = np.asarray(fp["norm_g"], np.float32)
        L["w1"] = np.asarray(fp["w1"], np.float32) * g[:, None]
        L["w2"] = np.asarray(fp["w2"], np.float32)
        out.append(L)
    return out



WNAMES_MM = ["wq", "wqs", "wk", "wks", "wv", "wo_self", "xwq", "xwk", "xwv",
             "wo_cross", "cwqkv", "wo_chan", "w1", "w2"]


def build_program():
    nc = bacc.Bacc(None, target_bir_lowering=False, num_devices=NCORES)
    groups = [[0, 1], [2, 3], [4, 5], [6, 7]]

    din0 = CFGS[0]["self_in"]
    T = {}
    T["x_in"] = nc.dram_tensor("x_in", [R, din0], F32, kind="ExternalInput")
    T["ctx_in"] = nc.dram_tensor("ctx_in", [M, PFD], F32, kind="ExternalInput")
    T["y_out"] = nc.dram_tensor("y_out", [R, DLAST], F32, kind="ExternalOutput")
    T["rev_in"] = nc.dram_tensor("rev_in", [P, P], F32, kind="ExternalInput")
    T["ident_in"] = nc.dram_tensor("ident_in", [P, P], F32, kind="ExternalInput")
    T["mask_in"] = nc.dram_tensor("mask_in", [2], F32, kind="ExternalInput")
    T["cosq_in"] = nc.dram_tensor("cosq_in", [P, R], F32, kind="ExternalInput")
    T["sinq_in"] = nc.dram_tensor("sinq_in", [P, R], F32, kind="ExternalInput")
    T["cosk_in"] = nc.dram_tensor("cosk_in", [DIM_HEAD, R], F32, kind="ExternalInput")
    T["sink_in"] = nc.dram_tensor("sink_in", [DIM_HEAD, R], F32, kind="ExternalInput")
    T["bias_in"] = nc.dram_tensor("bias_in", [HEADS, 2, 2048], F32, kind="ExternalInput")
    T["gfin_in"] = nc.dram_tensor("gfin_in", [DLAST], F32, kind="ExternalInput")
    T["dbg_out"] = nc.dram_tensor("dbg_out", [P, 8192], F32, kind="ExternalOutput")

    W = []
    for li, c in enumerate(CFGS):
        din, d, dco, dff = c["self_in"], c["d"], c["chan_out"], c["ff_d"]
        inner = FF_MULT * dff
        shp = dict(wq=[din, INNER], wqs=[din, INNER], wk=[din, DIM_HEAD],
                   wks=[din, DIM_HEAD], wv=[din, DIM_HEAD], wo_self=[INNER, d],
                   xwq=[d, INNER], xwk=[PFD, DIM_HEAD], xwv=[PFD, DIM_HEAD],
                   wo_cross=[INNER, d], cwqkv=[d, 3 * INNER], wo_chan=[INNER, dco],
                   w1=[dff, 2 * inner], w2=[inner, dff],
                   null_k=[DIM_HEAD], null_v=[DIM_HEAD],
                   xnull_k=[DIM_HEAD], xnull_v=[DIM_HEAD],
                   g_self_out=[d], g_cross_out=[d], g_chan_out=[dco])
        wl = {nm: nc.dram_tensor(f"L{li}_{nm}", s,
                                 MMDT if nm in WNAMES_MM else F32,
                                 kind="ExternalInput")
              for nm, s in shp.items()}
        W.append(wl)
    T["W"] = W

    KVLEN = 64 * R + R * 64
    T["KVLEN"] = KVLEN
    T["kv_cc_in"] = nc.dram_tensor("kv_cc_in", [2, KVLEN], F32)
    T["kv_cc_out"] = nc.dram_tensor("kv_cc_out", [2, KVLEN], F32)
    T["ch_cc_in"] = nc.dram_tensor("ch_cc_in", [2, 64 * HEADS * 64], F32)
    T["ch_cc_out"] = nc.dram_tensor("ch_cc_out", [2, 64 * HEADS * 64], F32)

    import contextlib
    with tile.TileContext(nc) as tc:
        with contextlib.ExitStack() as ctx:
            _build_body(ctx, nc, tc, groups, T)
    nc.compile()
    return nc


def _build_body(ctx, nc, tc, groups, T):
    W = T["W"]
    KVLEN = T["KVLEN"]
    F32R = os.environ.get("KB_F32R", "0") == "1"
    FR = mybir.dt.float32r

    def mm(out, lhsT, rhs, **kw):
        if F32R:
            lhsT = lhsT.bitcast(FR)
            rhs = rhs.bitcast(FR)
        nc.tensor.matmul(out, lhsT, rhs, **kw)
    DBG = os.environ.get("KB_DEBUG", "")
    dbg_written = [False]

    def dbg_dump(nc_, name, tiles):
        if name != DBG or dbg_written[0]:
            return
        dbg_written[0] = True
        off = 0
        for t, ncols in tiles:
            nparts = t.shape[0]
            nc_.sync.dma_start(
                out=T["dbg_out"][0:nparts, off:off + ncols], in_=t)
            off += ncols

    def pool(name, bufs, space="SBUF"):
        return ctx.enter_context(tc.tile_pool(name=name, bufs=bufs, space=space))

    const = pool("const", 1)
    persist = pool("persist", 2)
    xpool = pool("xpool", 5)
    xnpool = pool("xnpool", 4)
    ctxp = pool("ctxp", 2)
    zp = pool("zp", 4)
    xtp = pool("xtp", 6)
    xtrev = pool("xtrev", 6)
    qkp = pool("qkp", 5)
    vkm = pool("vkm", 3)
    vtp = pool("vtp", 6)
    kvp = pool("kvp", 1)
    vap = pool("vap", 1)
    ep = pool("ep", 5)
    bp = pool("bp", 2)
    otp = pool("otp", 8)
    onp = pool("onp", 2)
    wpool = pool("wpool", 4)
    gp = pool("gp", 2)
    spool = pool("spool", 3)
    denp = pool("denp", 2)
    chqk = pool("chqk", 2)
    chvp = pool("chvp", 3)
    chs = pool("chs", 2)
    ffp = pool("ffp", 8)
    ps_mm = pool("ps_mm", 2, "PSUM")
    ps_tr2 = pool("ps_tr2", 2, "PSUM")
    ps_av = pool("ps_av", 4, "PSUM")

    def bcast_vec(dst_pool, dram, dlen, tag, parts=P):
        t = dst_pool.tile([parts, dlen], F32, tag=tag)
        a = dram[:]
        nc.sync.dma_start(out=t, in_=bass.AP(tensor=a.tensor, offset=0,
                                             ap=[[0, parts], [1, dlen]]))
        return t

    rev_t = const.tile([P, P], F32)
    nc.sync.dma_start(out=rev_t, in_=T["rev_in"][:])
    ident_t = const.tile([P, P], F32)
    nc.sync.dma_start(out=ident_t, in_=T["ident_in"][:])
    ma = T["mask_in"][:]
    m0 = const.tile([P, 1], F32)
    m1 = const.tile([P, 1], F32)
    nc.sync.dma_start(out=m0, in_=bass.AP(tensor=ma.tensor, offset=0, ap=[[0, P], [1, 1]]))
    nc.sync.dma_start(out=m1, in_=bass.AP(tensor=ma.tensor, offset=1, ap=[[0, P], [1, 1]]))
    cosq = const.tile([P, R], F32)
    nc.sync.dma_start(out=cosq, in_=T["cosq_in"][:])
    sinq = const.tile([P, R], F32)
    nc.sync.dma_start(out=sinq, in_=T["sinq_in"][:])
    cosk = const.tile([DIM_HEAD, R], F32)
    nc.sync.dma_start(out=cosk, in_=T["cosk_in"][:])
    sink = const.tile([DIM_HEAD, R], F32)
    nc.sync.dma_start(out=sink, in_=T["sink_in"][:])
    magic = const.tile([P, 16], I32)
    nc.vector.memset(magic, 0x5f3759df)
    ones_t = const.tile([P, P], F32)
    nc.vector.memset(ones_t, 1.0)

    biasap = T["bias_in"][:]

    def bias_src(h, kind, off, n):
        return bass.AP(tensor=biasap.tensor, offset=h * 4096 + kind * 2048 + off,
                       ap=[[1, P], [1, n]])

    def copy_any(dst, src, use_act=True):
        if use_act:
            nc.scalar.copy(out=dst, in_=src)
        else:
            nc.vector.tensor_copy(out=dst, in_=src)

    def evict(dst, src_psum, eng=None):
        if eng is nc.vector:
            nc.vector.tensor_copy(out=dst, in_=src_psum)
        else:
            nc.scalar.copy(out=dst, in_=src_psum)

    def rsqrt_newton(v):
        n = v.shape[1]
        r = spool.tile([P, n], F32, tag="nrt_r")
        iv = r.bitcast(I32)
        nc.vector.tensor_scalar(out=iv, in0=v.bitcast(I32), scalar1=1,
                                scalar2=None, op0=ALU.arith_shift_right)
        nc.vector.tensor_tensor(out=iv, in0=magic[:, 0:n], in1=iv, op=ALU.subtract)
        t = spool.tile([P, n], F32, tag="nrt_t")
        for _ in range(3):
            nc.vector.tensor_tensor(out=t, in0=r, in1=r, op=ALU.mult)
            nc.vector.tensor_tensor(out=t, in0=t, in1=v, op=ALU.mult)
            nc.vector.tensor_scalar(out=t, in0=t, scalar1=-0.5, scalar2=1.5,
                                    op0=ALU.mult, op1=ALU.add)
            nc.vector.tensor_tensor(out=r, in0=r, in1=t, op=ALU.mult)
        nc.vector.tensor_copy(out=v, in_=r)
        return v

    def layernorm(tiles, d, out_pool, tag):
        nt = len(tiles)
        sub = math.gcd(512, d)
        nsub = d // sub
        var = spool.tile([P, nt], F32, tag="var")
        mean = spool.tile([P, nt], F32, tag="mean")
        for i, t in enumerate(tiles):
            stats = spool.tile([P, nsub, 6], F32, tag="bnstats")
            src3 = t.rearrange("p (s q) -> p s q", s=nsub)
            for s in range(nsub):
                nc.vector.bn_stats(out=stats[:, s, :], in_=src3[:, s, :])
            mv = spool.tile([P, 2], F32, tag="mv")
            nc.vector.bn_aggr(out=mv, in_=stats)
            nc.vector.tensor_copy(out=mean[:, i:i + 1], in_=mv[:, 0:1])
            nc.vector.tensor_copy(out=var[:, i:i + 1], in_=mv[:, 1:2])
        nc.vector.tensor_scalar_add(out=var, in0=var, scalar1=EPS)
        rstd = rsqrt_newton(var)
        nmr = spool.tile([P, nt], F32, tag="nmr")
        nc.vector.tensor_tensor(out=nmr, in0=mean, in1=rstd, op=ALU.mult)
        nc.vector.tensor_scalar_mul(out=nmr, in0=nmr, scalar1=-1.0)
        outs = []
        for i, t in enumerate(tiles):
            o = t if out_pool is None else out_pool.tile([P, d], F32, tag=tag)
            nc.scalar.activation(out=o, in_=t, func=AF.Identity,
                                 bias=nmr[:, i:i + 1], scale=rstd[:, i:i + 1])
            outs.append(o)
        return outs

    def transpose_to_feature(tiles, d, tag, reverse=False):
        nt = len(tiles)
        nkt = d // P
        dst_pool = xtrev if reverse else xtp
        perm = rev_t if reverse else ident_t
        outs = []
        for kt in range(nkt):
            ot = dst_pool.tile([P, nt * P], F32, tag=tag)
            for rt in range(nt):
                src = tiles[nt - 1 - rt] if reverse else tiles[rt]
                pst = ps_mm.tile([P, P], F32, tag="mm")
                nc.tensor.transpose(pst, src[:, kt * P:(kt + 1) * P], perm)
                copy_any(ot[:, rt * P:(rt + 1) * P], pst, use_act=(rt + kt) % 2 == 0)
            outs.append(ot)
        return outs

    def wslice(wdram, kt, c0, c1):
        t = wpool.tile([P, c1 - c0], MMDT, tag="w")
        nc.sync.dma_start(out=t, in_=wdram[kt * P:(kt + 1) * P, c0:c1])
        return t

    def feat_mm(psum, wdram, nkt, c0, c1, rhs_tiles):
        for i in range(nkt):
            mm(psum, wslice(wdram, i, c0, c1), rhs_tiles[i],
                             start=(i == 0), stop=(i == nkt - 1))

    din0 = CFGS[0]["self_in"]
    x_tiles = []
    for rt in range(4):
        xt0 = xpool.tile([P, din0], F32, tag="x")
        nc.sync.dma_start(out=xt0, in_=T["x_in"][rt * P:(rt + 1) * P, :])
        x_tiles.append(xt0)

    ctxnT = []
    for kt in range(2):
        ot = persist.tile([P, 8 * P], F32, tag="ctxnT")
        ctxnT.append(ot)
    for c0 in range(0, 8, 2):
        pairtiles = []
        for ct in (c0, c0 + 1):
            ctt = ctxp.tile([P, PFD], F32, tag="ctxload")
            nc.sync.dma_start(out=ctt, in_=T["ctx_in"][ct * P:(ct + 1) * P, :])
            pairtiles.append(ctt)
        pairn = layernorm(pairtiles, PFD, None, None)
        for j, ct in enumerate((c0, c0 + 1)):
            for kt in range(2):
                pst = ps_mm.tile([P, P], F32, tag="mm")
                nc.tensor.transpose(pst, pairn[j][:, kt * P:(kt + 1) * P], ident_t)
                copy_any(ctxnT[kt][:, ct * P:(ct + 1) * P], pst,
                         use_act=(ct + kt) % 2 == 0)

    def attention_core(qts, kT2, vaug, with_bias):
        outT = []
        for grp in range(2):
            av = []
            for _avi in range(4):
                avt = ps_av.tile([DIM_HEAD + 1, R], F32, tag="av")
                av.append(avt)
            for jt in range(JT):
                for pairi in range(2):
                    qtile = qts[grp * 2 + pairi]
                    for sub in range(2):
                        h = grp * 4 + pairi * 2 + sub
                        pss = ps_mm.tile([P, R], F32, tag="mm")
                        mm(
                            pss,
                            kT2[sub * 64:sub * 64 + 64, jt * P:(jt + 1) * P],
                            qtile[sub * 64:sub * 64 + 64, :],
                            start=True, stop=True,
                            tile_position=(sub * 64, 0),
                        )
                        e = ep.tile([P, R], F32, tag="E")
                        nc.scalar.activation(out=e, in_=pss, func=AF.Exp)
                        if with_bias:
                            bt = bp.tile([P, R], F32, tag="bias")
                            if jt < 4:
                                bsrc = bias_src(h, 0, jt * P + 512, R)
                            elif jt < 8:
                                bsrc = bias_src(h, 0, jt * P - 512, R)
                            else:
                                bsrc = bias_src(h, 1, 0, R)
                            nc.sync.dma_start(out=bt, in_=bsrc)
                            nc.vector.tensor_tensor(out=e, in0=e, in1=bt, op=ALU.mult)
                        mm(av[h - grp * 4], vaug[:, jt, :], e,
                                         start=(jt == 0), stop=(jt == JT - 1))
            for ai in range(4):
                a0 = av[ai]
                den = denp.tile([P, R], F32, tag="den")
                nc.scalar.copy(out=den[64:65, :], in_=a0[64:65, :])
                nc.vector.reciprocal(out=den[64:65, :], in_=den[64:65, :])
                psb = ps_mm.tile([P, R], F32, tag="mm")
                mm(psb, ones_t[64:65, :], den[64:65, :],
                                 start=True, stop=True, tile_position=(64, 0))
                onum = onp.tile([DIM_HEAD, R], F32, tag="onum")
                nc.scalar.copy(out=onum, in_=a0[0:64, :])
                ot = otp.tile([DIM_HEAD, R], F32, tag="outT")
                nc.vector.tensor_tensor(out=ot, in0=onum, in1=psb[0:64, :],
                                        op=ALU.mult)
                outT.append(ot)
        return outT

    def wo_block(outT, wodram, dout, gdram, x_tiles_in, residual):
        gt = bcast_vec(gp, gdram, dout, "gout")
        new_x = []
        for rt in range(4):
            zrow = zp.tile([P, dout], F32, tag="zrow")
            for ch in range((dout + 511) // 512):
                c0, c1 = ch * 512, min(dout, ch * 512 + 512)
                psy = ps_mm.tile([P, c1 - c0], F32, tag="mm")
                for h in range(HEADS):
                    wt = wpool.tile([DIM_HEAD, c1 - c0], MMDT, tag="w")
                    nc.sync.dma_start(out=wt, in_=wodram[h * 64:(h + 1) * 64, c0:c1])
                    mm(psy, outT[h][:, rt * P:(rt + 1) * P], wt,
                                     start=(h == 0), stop=(h == HEADS - 1))
                evict(zrow[:, c0:c1], psy)
            zl = layernorm([zrow], dout, None, None)[0]
            nc.vector.tensor_tensor(out=zl, in0=zl, in1=gt, op=ALU.mult)
            nx = xpool.tile([P, dout], F32, tag="x")
            if residual:
                nc.vector.tensor_tensor(out=nx, in0=zl, in1=x_tiles_in[rt], op=ALU.add)
            else:
                nc.vector.tensor_copy(out=nx, in_=zl)
            new_x.append(nx)
        return new_x

    for li, c in enumerate(CFGS):
        lw = W[li]
        din, d, dco, dff = c["self_in"], c["d"], c["chan_out"], c["ff_d"]
        inner = FF_MULT * dff
        nkt = din // P

        xn = layernorm(x_tiles, din, xnpool, "xn")
        if li == 0:
            dbg_dump(nc, "xn0", [(t, din) for t in xn])
        xnT = transpose_to_feature(xn, din, "xnT")
        xnTrev = transpose_to_feature(xn, din, "xnTrev", reverse=True)
        if li == 0:
            dbg_dump(nc, "xnT0", [(t, 512) for t in xnT])
            dbg_dump(nc, "xnTrev0", [(t, 512) for t in xnTrev])

        qts = []
        for qt in range(4):
            psq = ps_mm.tile([P, R], F32, tag="mm")
            feat_mm(psq, lw["wq"], nkt, qt * P, qt * P + P, xnT)
            psqs = ps_tr2.tile([P, R], F32, tag="trq")
            feat_mm(psqs, lw["wqs"], nkt, qt * P, qt * P + P, xnT)
            t1 = qkp.tile([P, R], F32, tag="qrot")
            nc.vector.tensor_tensor(out=t1, in0=psq, in1=cosq, op=ALU.mult)
            t2 = qkp.tile([P, R], F32, tag="qrot")
            nc.vector.tensor_tensor(out=t2, in0=psqs, in1=sinq, op=ALU.mult)
            nc.vector.tensor_tensor(out=t1, in0=t1, in1=t2, op=ALU.add)
            qts.append(t1)
        if li == 0:
            dbg_dump(nc, "q0", [(t, R) for t in qts])

        psk = ps_mm.tile([DIM_HEAD, R], F32, tag="mm")
        feat_mm(psk, lw["wk"], nkt, 0, DIM_HEAD, xnTrev)
        psks = ps_tr2.tile([DIM_HEAD, R], F32, tag="trq")
        feat_mm(psks, lw["wks"], nkt, 0, DIM_HEAD, xnTrev)
        krot = vkm.tile([DIM_HEAD, R], F32, tag="kmask")
        nc.vector.tensor_tensor(out=krot, in0=psk, in1=cosk, op=ALU.mult)
        ktmp = vkm.tile([DIM_HEAD, R], F32, tag="kmask")
        nc.vector.tensor_tensor(out=ktmp, in0=psks, in1=sink, op=ALU.mult)
        nc.vector.tensor_tensor(out=krot, in0=krot, in1=ktmp, op=ALU.add)

        v_tiles = []
        for vt in range(4):
            psv = ps_mm.tile([P, DIM_HEAD], F32, tag="mm")
            for i in range(nkt):
                wt = wslice(lw["wv"], i, 0, DIM_HEAD)
                mm(psv, xnTrev[i][:, vt * P:(vt + 1) * P], wt,
                                 start=(i == 0), stop=(i == nkt - 1))
            sv = vtp.tile([P, DIM_HEAD], F32, tag="vtile")
            evict(sv, psv)
            v_tiles.append(sv)

        for slot, msk in ((0, m0), (1, m1)):
            km = vkm.tile([DIM_HEAD, R], F32, tag="kmask")
            nc.vector.tensor_scalar_mul(out=km, in0=krot, scalar1=msk[0:DIM_HEAD])
            nc.sync.dma_start(
                out=T["kv_cc_in"][slot, 0:64 * R].rearrange("(p f) -> p f", p=64),
                in_=km)
            for vt in range(4):
                vm = vtp.tile([P, DIM_HEAD], F32, tag="vtile")
                nc.vector.tensor_scalar_mul(out=vm, in0=v_tiles[vt], scalar1=msk)
                nc.sync.dma_start(
                    out=T["kv_cc_in"][slot, 64 * R + vt * P * 64:
                                      64 * R + (vt + 1) * P * 64]
                    .rearrange("(p f) -> p f", p=P),
                    in_=vm)
        nc.gpsimd.collective_compute(
            "AllReduce", ALU.add, replica_groups=groups,
            ins=[T["kv_cc_in"][:]], outs=[T["kv_cc_out"][:]])

        kT2 = kvp.tile([P, JT * P], F32, tag="kT2")
        nc.vector.memset(kT2[:, 8 * P:JT * P], 0.0)
        for half in range(2):
            for slot in range(2):
                nc.sync.dma_start(
                    out=kT2[half * 64:half * 64 + 64, slot * R:(slot + 1) * R],
                    in_=T["kv_cc_out"][slot, 0:64 * R].rearrange("(p f) -> p f", p=64))
            nka = lw["null_k"][:]
            nc.sync.dma_start(
                out=kT2[half * 64:half * 64 + 64, 8 * P:8 * P + 1],
                in_=bass.AP(tensor=nka.tensor, offset=0, ap=[[1, 64], [1, 1]]))
        vaug = vap.tile([P, JT, DIM_HEAD + 1], F32, tag="vaug")
        nc.vector.memset(vaug[:, 8, :], 0.0)
        nc.vector.memset(vaug[:, 0:8, 64:65], 1.0)
        for slot in range(2):
            nc.sync.dma_start(
                out=vaug[:, slot * 4:slot * 4 + 4, 0:64],
                in_=T["kv_cc_out"][slot, 64 * R:KVLEN]
                .rearrange("(t p f) -> p t f", p=P, t=4))
        nva = lw["null_v"][:]
        nc.sync.dma_start(out=vaug[0:1, 8, 0:64],
                          in_=bass.AP(tensor=nva.tensor, offset=0, ap=[[0, 1], [1, 64]]))
        nc.vector.memset(vaug[0:1, 8, 64:65], 1.0)
        if li == 0:
            dbg_dump(nc, "kT2", [(kT2, JT * P)])
            dbg_dump(nc, "vaug", [(vaug.rearrange("p t f -> p (t f)"), JT * 65)])
            dbg_dump(nc, "krot", [(krot, R)])

        outT = attention_core(qts, kT2, vaug, with_bias=True)
        if li == 0:
            dbg_dump(nc, "sa_outT", [(t, R) for t in outT])
        x_tiles = wo_block(outT, lw["wo_self"], d, lw["g_self_out"], x_tiles,
                           residual=(li != 0))
        if li == 0:
            dbg_dump(nc, "x_sa", [(t, d) for t in x_tiles])

        xn = layernorm(x_tiles, d, xnpool, "xn")
        xnT = transpose_to_feature(xn, d, "xnT")
        ndkt = d // P
        qts = []
        for qt in range(4):
            psq = ps_mm.tile([P, R], F32, tag="mm")
            feat_mm(psq, lw["xwq"], ndkt, qt * P, qt * P + P, xnT)
            t1 = qkp.tile([P, R], F32, tag="qrot")
            evict(t1, psq)
            qts.append(t1)
        kT2x = kvp.tile([P, JT * P], F32, tag="kT2")
        nc.vector.memset(kT2x[:, 8 * P:JT * P], 0.0)
        for nch in range(2):
            pskx = ps_mm.tile([DIM_HEAD, 512], F32, tag="mm")
            for i in range(2):
                wt = wslice(lw["xwk"], i, 0, DIM_HEAD)
                mm(pskx, wt, ctxnT[i][:, nch * 512:(nch + 1) * 512],
                                 start=(i == 0), stop=(i == 1))
            evict(kT2x[0:64, nch * 512:(nch + 1) * 512], pskx)
        nka = lw["xnull_k"][:]
        nc.sync.dma_start(
            out=kT2x[0:64, 8 * P:8 * P + 1],
            in_=bass.AP(tensor=nka.tensor, offset=0, ap=[[1, 64], [1, 1]]))
        nc.sync.dma_start(out=kT2x[64:128, :], in_=kT2x[0:64, :])
        vaugx = vap.tile([P, JT, DIM_HEAD + 1], F32, tag="vaug")
        nc.vector.memset(vaugx[:, 8, :], 0.0)
        nc.vector.memset(vaugx[:, 0:8, 64:65], 1.0)
        for jt in range(8):
            psvx = ps_mm.tile([P, DIM_HEAD], F32, tag="mm")
            for i in range(2):
                wt = wslice(lw["xwv"], i, 0, DIM_HEAD)
                mm(psvx, ctxnT[i][:, jt * P:(jt + 1) * P], wt,
                                 start=(i == 0), stop=(i == 1))
            evict(vaugx[:, jt, 0:64], psvx, eng=nc.vector)
        nva = lw["xnull_v"][:]
        nc.sync.dma_start(out=vaugx[0:1, 8, 0:64],
                          in_=bass.AP(tensor=nva.tensor, offset=0, ap=[[0, 1], [1, 64]]))
        nc.vector.memset(vaugx[0:1, 8, 64:65], 1.0)

        outT = attention_core(qts, kT2x, vaugx, with_bias=False)
        x_tiles = wo_block(outT, lw["wo_cross"], d, lw["g_cross_out"], x_tiles,
                           residual=True)
        if li == 0:
            dbg_dump(nc, "x_cross", [(t, d) for t in x_tiles])

        xn = layernorm(x_tiles, d, xnpool, "xn")
        xnT = transpose_to_feature(xn, d, "xnT")
        sims = chs.tile([64, HEADS, 64], F32, tag="chsim")
        for rt in range(4):
            psq = ps_mm.tile([P, INNER], F32, tag="mm")
            for i in range(ndkt):
                wt = wslice(lw["cwqkv"], i, 0, INNER)
                mm(psq, xnT[i][:, rt * P:(rt + 1) * P], wt,
                                 start=(i == 0), stop=(i == ndkt - 1))
            tq = chqk.tile([P, INNER], F32, tag="chq")
            evict(tq, psq)
            psk2 = ps_mm.tile([P, INNER], F32, tag="mm")
            for i in range(ndkt):
                wt = wslice(lw["cwqkv"], i, INNER, 2 * INNER)
                mm(psk2, xnT[i][:, rt * P:(rt + 1) * P], wt,
                                 start=(i == 0), stop=(i == ndkt - 1))
            tk = chqk.tile([P, INNER], F32, tag="chk")
            evict(tk, psk2, eng=nc.vector)
            for h in range(HEADS):
                pss = ps_mm.tile([64, 64], F32, tag="mm")
                mm(pss, tq[:, h * 64:(h + 1) * 64],
                                 tk[:, h * 64:(h + 1) * 64],
                                 start=True, stop=True)
                if rt == 0:
                    evict(sims[:, h, :], pss, eng=nc.vector if h % 2 else None)
                else:
                    nc.vector.tensor_tensor(out=sims[:, h, :], in0=sims[:, h, :],
                                            in1=pss, op=ALU.add)
        for slot, msk in ((0, m0), (1, m1)):
            sm = chs.tile([64, HEADS, 64], F32, tag="chsimm")
            nc.vector.tensor_scalar_mul(out=sm, in0=sims, scalar1=msk[0:64])
            nc.sync.dma_start(
                out=T["ch_cc_in"][slot].rearrange("(p f) -> p f", p=64),
                in_=sm.rearrange("p h f -> p (h f)"))
        nc.gpsimd.collective_compute(
            "AllReduce", ALU.add, replica_groups=groups,
            ins=[T["ch_cc_in"][:]], outs=[T["ch_cc_out"][:]])
        simf = chs.tile([64, HEADS, 64], F32, tag="chsim")
        nc.sync.dma_start(
            out=simf.rearrange("p h f -> p (h f)"),
            in_=T["ch_cc_out"][0].rearrange("(p f) -> p f", p=64))
        sim2 = chs.tile([64, HEADS, 64], F32, tag="chsimm")
        nc.sync.dma_start(
            out=sim2.rearrange("p h f -> p (h f)"),
            in_=T["ch_cc_out"][1].rearrange("(p f) -> p f", p=64))
        nc.vector.tensor_tensor(out=simf, in0=simf, in1=sim2, op=ALU.add)
        mx = spool.tile([64, HEADS], F32, tag="chmx")
        nc.vector.tensor_reduce(out=mx, in_=simf, axis=mybir.AxisListType.X,
                                op=ALU.max, negate=True)
        att = chs.tile([64, HEADS, 64], F32, tag="chsimm")
        for h in range(HEADS):
            nc.scalar.activation(out=att[:, h, :], in_=simf[:, h, :], func=AF.Exp,
                                 bias=mx[:, h:h + 1], scale=1.0)
        ssum = spool.tile([64, HEADS], F32, tag="chsum")
        nc.vector.tensor_reduce(out=ssum, in_=att, axis=mybir.AxisListType.X,
                                op=ALU.add)
        nc.vector.reciprocal(out=ssum, in_=ssum)
        for h in range(HEADS):
            nc.vector.tensor_scalar_mul(out=att[:, h, :], in0=att[:, h, :],
                                        scalar1=ssum[:, h:h + 1])
        outT = []
        for h in range(HEADS):
            psv = ps_mm.tile([DIM_HEAD, R], F32, tag="mm")
            feat_mm(psv, lw["cwqkv"], ndkt, 2 * INNER + h * 64,
                    2 * INNER + (h + 1) * 64, xnT)
            tv = chvp.tile([DIM_HEAD, R], F32, tag="chv")
            evict(tv, psv)
            psT = ps_mm.tile([64, 64], F32, tag="mm")
            nc.tensor.transpose(psT, att[:, h, :], ident_t[0:64, 0:64])
            attT = spool.tile([64, 64], F32, tag="chattT")
            evict(attT, psT)
            pso = ps_mm.tile([64, R], F32, tag="mm")
            mm(pso, attT, tv, start=True, stop=True)
            ot = otp.tile([DIM_HEAD, R], F32, tag="outT")
            evict(ot, pso, eng=None if h % 2 else nc.vector)
            outT.append(ot)
        x_tiles = wo_block(outT, lw["wo_chan"], dco, lw["g_chan_out"], x_tiles,
                           residual=(li != DEPTH + 1))
        if li == 0:
            dbg_dump(nc, "x_chan", [(t, dco) for t in x_tiles])

        xn = layernorm(x_tiles, dff, xnpool, "xn")
        xnT = transpose_to_feature(xn, dff, "xnT")
        nffk = dff // P
        nI = inner // P
        HPASS = 6
        ysum = [None] * 4
        for base in range(0, nI, HPASS):
            cnt = min(HPASS, nI - base)
            gact = []
            for ii in range(cnt):
                i = base + ii
                psa = ps_mm.tile([P, R], F32, tag="mm")
                feat_mm(psa, lw["w1"], nffk, i * P, (i + 1) * P, xnT)
                psg = ps_tr2.tile([P, R], F32, tag="trq")
                feat_mm(psg, lw["w1"], nffk, inner + i * P, inner + (i + 1) * P, xnT)
                tg = ffp.tile([P, R], F32, tag="gact")
                th = ffp.tile([P, R], F32, tag="gact")
                nc.scalar.activation(out=th, in_=psg, func=AF.Tanh, scale=0.5)
                nc.vector.tensor_scalar(out=th, in0=th, scalar1=0.5, scalar2=0.5,
                                        op0=ALU.mult, op1=ALU.add)
                nc.vector.tensor_tensor(out=th, in0=th, in1=psg, op=ALU.mult)
                nc.vector.tensor_tensor(out=tg, in0=th, in1=psa, op=ALU.mult)
                gact.append(tg)
            for rt in range(4):
                for ch in range((dff + 511) // 512):
                    c0, c1 = ch * 512, min(dff, ch * 512 + 512)
                    psy = ps_mm.tile([P, c1 - c0], F32, tag="mm")
                    for ii in range(cnt):
                        i = base + ii
                        wt = wslice(lw["w2"], i, c0, c1)
                        mm(psy, gact[ii][:, rt * P:(rt + 1) * P], wt,
                                         start=(ii == 0), stop=(ii == cnt - 1))
                    if base == 0:
                        if ysum[rt] is None:
                            zr = zp.tile([P, dff], F32, tag="zrow")
                            ysum[rt] = zr
                        evict(ysum[rt][:, c0:c1], psy)
                    else:
                        nc.vector.tensor_tensor(out=ysum[rt][:, c0:c1],
                                                in0=ysum[rt][:, c0:c1], in1=psy,
                                                op=ALU.add)
        new_x = []
        for rt in range(4):
            nx = xpool.tile([P, dff], F32, tag="x")
            nc.vector.tensor_tensor(out=nx, in0=ysum[rt], in1=x_tiles[rt], op=ALU.add)
            new_x.append(nx)
        x_tiles = new_x

    gfin = bcast_vec(gp, T["gfin_in"], DLAST, "gout")
    xdiv = []
    for rt in range(4):
        mxv = spool.tile([P, 1], F32, tag="fmax")
        nc.vector.tensor_reduce(out=mxv, in_=x_tiles[rt], axis=mybir.AxisListType.X,
                                op=ALU.max)
        nc.vector.reciprocal(out=mxv, in_=mxv)
        td = xnpool.tile([P, DLAST], F32, tag="xn")
        nc.vector.tensor_scalar_mul(out=td, in0=x_tiles[rt], scalar1=mxv)
        xdiv.append(td)
    xfin = layernorm(xdiv, DLAST, None, None)
    for rt in range(4):
        nc.vector.tensor_tensor(out=xfin[rt], in0=xfin[rt], in1=gfin, op=ALU.mult)
        nc.sync.dma_start(out=T["y_out"][rt * P:(rt + 1) * P, :], in_=xfin[rt])



_CACHE = {}


def _get_nc():
    key = (NLAYERS, str(MMDT), os.environ.get("KB_DEBUG", ""), os.environ.get("KB_F32R", "0"))
    if key not in _CACHE:
        _CACHE[key] = build_program()
    return _CACHE[key]


def make_in_maps(x, context, params):
    x = np.asarray(x, np.float32)
    context = np.asarray(context, np.float32)
    folded = fold_weights(params)
    emb = np.asarray(params["rel_pos_emb"], np.float32)
    gfin = np.asarray(params["out_norm_g"], np.float32)[:DLAST]

    rev = np.zeros((P, P), np.float32)
    rev[np.arange(P), P - 1 - np.arange(P)] = 1.0
    ident = np.eye(P, dtype=np.float32)

    in_maps = []
    for core in range(NCORES):
        b = core // 2
        parity = core % 2
        cosq, sinq, cosk, sink = make_rotary(parity)
        im = {
            "x_in": np.ascontiguousarray(x[b, parity * R:(parity + 1) * R, :]),
            "ctx_in": np.ascontiguousarray(context[b]),
            "rev_in": rev, "ident_in": ident,
            "mask_in": np.array([1.0 - parity, float(parity)], np.float32),
            "cosq_in": cosq, "sinq_in": sinq, "cosk_in": cosk, "sink_in": sink,
            "bias_in": make_bias_tables(emb, parity),
            "gfin_in": gfin,
        }
        for li, L in enumerate(folded):
            for nm, v in L.items():
                im[f"L{li}_{nm}"] = np.ascontiguousarray(v)
        in_maps.append(im)
    return in_maps


def kernel(x, context, params):
    nc = _get_nc()
    in_maps = make_in_maps(x, context, params)
    res = run_bass_kernel_spmd(nc, in_maps, core_ids=list(range(NCORES)))
    out = np.zeros((B, N, DLAST), np.float32)
    for core in range(NCORES):
        b, parity = core // 2, core % 2
        out[b, parity * R:(parity + 1) * R, :] = res.results[core]["y_out"]
    return out
